# revision 1
# baseline (speedup 1.0000x reference)
"""Trainium2 Bass kernel for nn_AttentionBlock (B=4, S=2048, D=1024, single head).

Sharding: 8 cores = 4 batches x 2 query-halves; each core owns 1024 queries
of one batch and returns that [1024, 1024] slice of the output (transposed;
the host gather transposes it back).

Algebraic restructure with two weight-only folds (host-side, x-independent):
    W2 = Wk^T @ Wq   ->  scores = Q K^T = Xq Wq^T Wk Xk^T = (X W2 Xq^T)^T
    W3 = Wp @ Wv     ->  attn V Wp^T = attn (X W3^T)
so per core:
    G  [D, 1024]    = W2 @ Xq^T   (d-major, PE starts after ~1MB of input)
    scoresT / VP    = fused loop over 16 key tiles: each streamed 128x128
                      X^T stationary tile feeds 4 matmuls (2 scoresT chunks,
                      2 VP chunks); VP = X @ W3^T stays resident in SBUF
    expT = exp(scoresT * scale)   (no max subtraction; scores are O(1))
    rowsum r_row via ones-column matmul; r broadcast to [128, SQ] via a
    K=1 ones-row matmul (fp32)
    yT [D, 1024] = VP.T @ expT in four 2-tile f-passes (alternating PSUM
    tag pairs so normalize/store overlaps the next pass's matmuls), then
    yT * r_bcast + bias_col on VectorE, DMA out transposed.
Q, K, V, and the output projection never exist on the device.

Matmuls run in float32r (fp32 storage, reduced-precision PE multiply,
1 cycle/row vs fp32's 4). Keys are permuted per-core (own half first) -
attention is permutation invariant.

SBUF tags are aliased across phases (pool memory is the static sum over
tags): xq0..7 carry Xq^T -> VP[8..15]; g0..7 carry G -> r_row/r_bcast;
expT8..15 carry W2^T before scores reach them; w0..7 carry W3^T.
PSUM uses 4 double-bank tags q0..q3.
"""

import numpy as np
from contextlib import ExitStack

D = 1024
S = 2048
SQ = 1024  # queries per core
P = 128
SCALE = float(1.0 / np.sqrt(np.float32(D)).astype(np.float32))

_CACHED = {}


def _build_nc():
    import concourse.tile as tile
    from concourse import bacc, mybir

    DT = mybir.dt.float32
    F32R = mybir.dt.float32r
    FP = mybir.dt.float32
    Exp = mybir.ActivationFunctionType.Exp
    MUL = mybir.AluOpType.mult

    nc = bacc.Bacc("TRN2", target_bir_lowering=False)
    xt_d = nc.declare_dram_parameter("xt", [D, S], F32R, isOutput=False)
    w2t_d = nc.declare_dram_parameter("w2t", [D, D], F32R, isOutput=False)
    w3t_d = nc.declare_dram_parameter("w3t", [D, D], F32R, isOutput=False)
    biasc_d = nc.declare_dram_parameter("biasc", [P, 8], DT, isOutput=False)
    ones_d = nc.declare_dram_parameter("ones", [P, 1], F32R, isOutput=False)
    onesr_d = nc.declare_dram_parameter("onesr", [1, P], DT, isOutput=False)
    yt_d = nc.declare_dram_parameter("yt", [D, SQ], DT, isOutput=True)

    ND = D // P     # 8 tiles along D
    NS = S // P     # 16 tiles along S

    with tile.TileContext(nc) as tc:
        with ExitStack() as ctx:
            pool = ctx.enter_context(tc.tile_pool(name="main", bufs=1))
            psum = ctx.enter_context(tc.tile_pool(name="psum", bufs=1, space="PSUM"))

            def ptile(shape, name, tag, bufs=1, dt=F32R):
                return pool.tile(shape, dt, name=name, tag=tag, bufs=bufs)

            def qbank(i, name, shape=(P, 1024)):
                # 4 PSUM tags x 2 banks each = all 8 banks
                return psum.tile(list(shape), FP, name=name, tag=f"q{i}", bufs=1)

            # ---- resident inputs: W2^T + Xq^T first (phase-1 critical path)
            xq = []
            w2t = []
            for d in range(ND):
                t = ptile([P, D], f"w2t{d}", f"expT{8 + d}")
                if d < 2:  # halves so the first G matmul starts sooner
                    nc.sync.dma_start(t[:, 0:512], w2t_d[d * P:(d + 1) * P, 0:512])
                    nc.sync.dma_start(t[:, 512:1024],
                                      w2t_d[d * P:(d + 1) * P, 512:1024])
                else:
                    nc.sync.dma_start(t[:], w2t_d[d * P:(d + 1) * P, :])
                w2t.append(t)
                t = ptile([P, SQ], f"xq{d}", f"xq{d}")
                if d < 2:
                    nc.sync.dma_start(t[:, 0:512], xt_d[d * P:(d + 1) * P, 0:512])
                    nc.sync.dma_start(t[:, 512:1024],
                                      xt_d[d * P:(d + 1) * P, 512:1024])
                else:
                    nc.sync.dma_start(t[:], xt_d[d * P:(d + 1) * P, 0:SQ])
                xq.append(t)
            w3t = []
            for d in range(ND):
                t = ptile([P, D], f"w{d}", f"w{d}")
                nc.sync.dma_start(t[:], w3t_d[d * P:(d + 1) * P, :])
                w3t.append(t)
            ones_sb = ptile([P, 1], "ones", "ones")
            nc.sync.dma_start(ones_sb[:], ones_d[:, :])
            onesr_sb = ptile([1, P], "onesr", "onesr", dt=DT)
            nc.sync.dma_start(onesr_sb[:], onesr_d[:, :])
            biasc_sb = ptile([P, 8], "biasc", "biasc", dt=DT)
            nc.sync.dma_start(biasc_sb[:], biasc_d[:, :])

            # ---- phase 1: G[g][128, SQ] = sum_d w2t[d][:, g].T @ xq[d] ----
            g_sb = []
            for g in range(ND):
                g_sb.append(ptile([P, SQ], f"g{g}", f"g{g}"))
            for gp in range(2):  # d-major so the PE starts after ~1MB of input
                pgs = [qbank(i, f"pg_{gp}_{i}") for i in range(4)]
                for d in range(ND):
                    for i in range(4):
                        g = gp * 4 + i
                        lt = w2t[d][:, g * P:(g + 1) * P]
                        nc.tensor.matmul(pgs[i][:, 0:512], lt, xq[d][:, 0:512],
                                         start=(d == 0), stop=(d == ND - 1))
                        nc.tensor.matmul(pgs[i][:, 512:1024], lt,
                                         xq[d][:, 512:1024],
                                         start=(d == 0), stop=(d == ND - 1))
                for i in range(4):
                    g = gp * 4 + i
                    nc.vector.tensor_copy(g_sb[g][:, 0:512], pgs[i][:, 0:512])
                    nc.vector.tensor_copy(g_sb[g][:, 512:1024], pgs[i][:, 512:1024])

            # ---- phase 2 (fused): per key tile sk, stream the X^T stationary
            #      tile once; 2 matmuls for scoresT (-> exp -> expT) and 2 for
            #      VP = X @ W3^T. VP[sk] stays resident in SBUF. ----
            expT = []
            for sk in range(NS):
                expT.append(ptile([P, SQ], f"expT{sk}", f"expT{sk}"))
            vp = []
            for sk in range(NS):
                tag = f"vres{sk}" if sk < 8 else f"xq{sk - 8}"
                vp.append(ptile([P, D], f"vp{sk}", tag))
            for sk in range(NS):
                psc = qbank(sk % 2, f"psc_{sk}")
                ps0, ps1 = psc[:, 0:512], psc[:, 512:1024]
                pv = qbank(2 + sk % 2, f"pv_{sk}")
                pv0, pv1 = pv[:, 0:512], pv[:, 512:1024]
                for d in range(ND):
                    if sk < 8:
                        # first 8 key tiles == query columns, already resident
                        xs_ap = xq[d][:, sk * P:(sk + 1) * P]
                    else:
                        xs = ptile([P, P], f"xs_{sk}_{d}", "xs", bufs=16)
                        nc.sync.dma_start(
                            xs[:], xt_d[d * P:(d + 1) * P, sk * P:(sk + 1) * P])
                        xs_ap = xs[:]
                    nc.tensor.matmul(ps0, xs_ap, g_sb[d][:, 0:512],
                                     start=(d == 0), stop=(d == ND - 1))
                    nc.tensor.matmul(ps1, xs_ap, g_sb[d][:, 512:1024],
                                     start=(d == 0), stop=(d == ND - 1))
                    nc.tensor.matmul(pv0, xs_ap, w3t[d][:, 0:512],
                                     start=(d == 0), stop=(d == ND - 1))
                    nc.tensor.matmul(pv1, xs_ap, w3t[d][:, 512:1024],
                                     start=(d == 0), stop=(d == ND - 1))
                nc.scalar.activation(expT[sk][:, 0:512], ps0, Exp, scale=SCALE)
                nc.scalar.activation(expT[sk][:, 512:1024], ps1, Exp, scale=SCALE)
                nc.vector.tensor_copy(vp[sk][:, 0:512], pv0)
                nc.vector.tensor_copy(vp[sk][:, 512:1024], pv1)
                # running key-tile sum for the softmax rowsum (hidden on DVE)
                if sk == 0:
                    acc_sb = ptile([P, SQ], "acc_sb", "acc_sb")
                    nc.vector.tensor_copy(acc_sb[:], expT[0][:])
                else:
                    nc.vector.tensor_tensor(acc_sb[:], acc_sb[:], expT[sk][:],
                                            mybir.AluOpType.add)

            # ---- phase 3: rowsum -> r_bcast [128, SQ] ----
            pc = qbank(2, "pcs", shape=(1, 1024))
            pc0, pc1 = pc[0:1, 0:512], pc[0:1, 512:1024]
            nc.tensor.matmul(pc0, ones_sb[:], acc_sb[:, 0:512],
                             start=True, stop=True)
            nc.tensor.matmul(pc1, ones_sb[:], acc_sb[:, 512:1024],
                             start=True, stop=True)
            r_row = ptile([1, SQ], "r_row", "g0", dt=DT)
            nc.vector.reciprocal(r_row[0:1, 0:512], pc0)
            nc.vector.reciprocal(r_row[0:1, 512:1024], pc1)
            rb_sb = ptile([P, SQ], "rb_sb", "g1", dt=DT)

            # ---- phase 4: yT[f][128, SQ] = sum_sk VP[sk][:, f].T @ expT[sk],
            #      four 2-tile passes (finals overlap the next pass's
            #      matmuls); normalize + bias on VectorE; DMA out. The r
            #      broadcast matmuls are emitted after fg0's matmuls so the
            #      in-order PE never waits on the reciprocal. ----
            FGROUPS = [(0, 2), (2, 2), (4, 2), (6, 1), (7, 1)]
            qrot = 0
            for fg, (fbase, fcnt) in enumerate(FGROUPS):
                otp = [qbank((qrot + i) % 4, f"ot_{fg}_{i}") for i in range(fcnt)]
                qrot = (qrot + fcnt) % 4
                for sk in range(NS):
                    for i in range(fcnt):
                        f = fbase + i
                        lt = vp[sk][:, f * P:(f + 1) * P]
                        nc.tensor.matmul(otp[i][:, 0:512], lt,
                                         expT[sk][:, 0:512],
                                         start=(sk == 0), stop=(sk == NS - 1))
                        nc.tensor.matmul(otp[i][:, 512:1024], lt,
                                         expT[sk][:, 512:1024],
                                         start=(sk == 0), stop=(sk == NS - 1))
                if fg == 0:
                    # broadcast r_row across partitions via K=1 fp32 matmul
                    prb = qbank(3, "prb")
                    nc.tensor.matmul(prb[:, 0:512], onesr_sb[:],
                                     r_row[0:1, 0:512], start=True, stop=True)
                    nc.tensor.matmul(prb[:, 512:1024], onesr_sb[:],
                                     r_row[0:1, 512:1024], start=True, stop=True)
                    nc.vector.tensor_copy(rb_sb[:, 0:512], prb[:, 0:512])
                    nc.vector.tensor_copy(rb_sb[:, 512:1024], prb[:, 512:1024])
                for i in range(fcnt):
                    f = fbase + i
                    ysb = ptile([P, SQ], f"ysb_{f}", f"g{2 + f % 4}", dt=DT)
                    nc.vector.tensor_tensor(ysb[:], otp[i][:], rb_sb[:], MUL)
                    nc.vector.tensor_scalar_add(ysb[:], ysb[:],
                                                biasc_sb[:, f:f + 1])
                    nc.sync.dma_start(yt_d[f * P:(f + 1) * P, :], ysb[:])

    nc.compile()
    return nc


def _get_nc():
    if "nc" not in _CACHED:
        _CACHED["nc"] = _build_nc()
    return _CACHED["nc"]


def make_in_maps(x, w_qkv, w_proj, b_proj):
    wq = w_qkv[0:D]
    wk = w_qkv[D:2 * D]
    wv = w_qkv[2 * D:3 * D]
    w2 = wk.T @ wq                   # scores = X W2 Xq^T
    w3 = w_proj @ wv                 # attn V Wp^T = attn (X W3^T)
    w2T = np.ascontiguousarray(w2.T)
    w3T = np.ascontiguousarray(w3.T)
    biasc = np.ascontiguousarray(b_proj.reshape(8, P).T)
    ones = np.ones((P, 1), dtype=np.float32)
    onesr = np.ones((1, P), dtype=np.float32)
    in_maps = []
    for c in range(8):
        b, h = c // 2, c % 2
        own = x[b, h * SQ:(h + 1) * SQ]       # [1024, D] our queries
        other = x[b, (1 - h) * SQ:(2 - h) * SQ]
        xt = np.ascontiguousarray(np.concatenate([own.T, other.T], axis=1))
        in_maps.append({
            "xt": xt, "w2t": w2T, "w3t": w3T,
            "biasc": biasc, "ones": ones, "onesr": onesr,
        })
    return in_maps


def gather_out(results):
    out = np.empty((4, S, D), dtype=np.float32)
    for c in range(8):
        b, h = c // 2, c % 2
        out[b, h * SQ:(h + 1) * SQ] = results[c]["yt"].T
    return out


def kernel(x, w_qkv, w_proj, b_proj):
    from concourse import bass_utils
    nc = _get_nc()
    in_maps = make_in_maps(np.asarray(x, dtype=np.float32),
                           np.asarray(w_qkv, dtype=np.float32),
                           np.asarray(w_proj, dtype=np.float32),
                           np.asarray(b_proj, dtype=np.float32))
    res = bass_utils.run_bass_kernel_spmd(nc, in_maps, list(range(8))).results
    return gather_out(res)



# revision 3
# speedup vs baseline: 1.5840x; 1.5840x over previous
"""Trainium2 Bass kernel for nn_AttentionBlock (B=4, S=2048, D=1024, single head).

Sharding: 8 cores = 4 batches x 2 query-halves; each core owns 1024 queries
of one batch and returns that [1024, 1024] slice of the output (transposed;
the host gather transposes it back).

Algebraic restructure (all folds host-side, x-independent):
    W2 = Wk^T @ Wq   ->  scoresT = Xk W2 Xq^T   (Q, K never materialized)
    y  = softmax(s) @ V @ Wp^T = (E @ X) @ W3^T / rowsum,  W3 = Wp @ Wv
The projection is applied AFTER the attention-weighted sum of X (z = E @ X,
then y = z @ W3^T). This is cheaper than the VP = X @ W3^T route because z
is per-core-unique while VP would be recomputed by both cores of a batch.

Precision: the scores path (G = W2' Xq^T and scoresT = Xk G) runs in
fp8 e4m3 with DoubleRow perf mode (2 K-tiles per matmul, ~1.4x PE
throughput); softmax is insensitive to ~1% score noise. W2 is pre-scaled by
ALPHA=64 so its entries are normal-range in fp8; the exp activation scale
absorbs 1/ALPHA. The output path (zT = X^T E^T, yT = W3 zT) runs in
fp16 operands with fp32 PSUM accumulation (~5e-4 relative error).

Phases (PE-major, in-order engines; PSUM = 4 double-bank tags q0..q3):
    1. G[d][128, 1024q] = sum_e W2'^T[e-pairs] @ Xq^T[e-pairs]  (fp8 DR)
       -> cast to g8 d-pair tiles (DVE + ACT split)
    2. per key tile sk: scoresT = sum_d Xk^T[d-pairs] @ g8      (fp8 DR)
       -> exp (ACT, scale=SCALE/ALPHA) -> expT fp16; DVE running rowsum
    3. rowsum via ones-column matmul -> reciprocal -> r broadcast via
       K=1 f32r matmul (emitted inside phase 4's first group)
    4. zT[f][128d, 1024q] = sum_sk Xrow[sk,f-cols] @ expT[sk]   (fp16)
       in 2-tile f-passes with rotating PSUM tags -> zt_sb fp16
    5. yT[e][128, 1024q] = sum_d W3^T[d][e-cols] @ zt_sb[d]     (fp16)
       -> DVE: * r_bcast + bias -> DMA out transposed
"""

import numpy as np
import ml_dtypes
from contextlib import ExitStack

D = 1024
S = 2048
SQ = 1024  # queries per core
P = 128
ALPHA = 64.0  # host pre-scale on W2 so fp8 e4m3 stays in normal range
SCALE = float(1.0 / np.sqrt(np.float32(D)).astype(np.float32))
ESCALE = SCALE / ALPHA

_CACHED = {}


def _build_nc():
    import concourse.tile as tile
    from concourse import bacc, mybir

    FP = mybir.dt.float32
    F32R = mybir.dt.float32r
    F16 = mybir.dt.float16
    F8 = mybir.dt.float8e4
    Exp = mybir.ActivationFunctionType.Exp
    Copy = mybir.ActivationFunctionType.Copy
    MUL = mybir.AluOpType.mult
    ADD = mybir.AluOpType.add
    DR = mybir.MatmulPerfMode.DoubleRow

    nc = bacc.Bacc("TRN2", target_bir_lowering=False)
    # x feature-major fp8 pair tiles: [dd, p, i, k] = xp[k, 256*dd+128*i+p]
    xall8_d = nc.declare_dram_parameter("xall8", [4, P, 2, S], F8, isOutput=False)
    # W2'^T pair tiles: [ee, p, i, d] = ALPHA*w2[d, 256*ee+128*i+p]
    w2t8_d = nc.declare_dram_parameter("w2t8", [4, P, 2, D], F8, isOutput=False)
    # x row-major fp16: [sk, p, d] = xp[128*sk+p, d]
    xrow_d = nc.declare_dram_parameter("xrow", [16, P, D], F16, isOutput=False)
    # W3^T fp16: [dt, p, e] = w3[e, 128*dt+p]
    w3t_d = nc.declare_dram_parameter("w3t", [8, P, D], F16, isOutput=False)
    biasc_d = nc.declare_dram_parameter("biasc", [P, 8], FP, isOutput=False)
    ones_d = nc.declare_dram_parameter("ones", [P, 1], F32R, isOutput=False)
    onesr_d = nc.declare_dram_parameter("onesr", [1, P], F32R, isOutput=False)
    yt_d = nc.declare_dram_parameter("yt", [D, SQ], FP, isOutput=True)

    ND = D // P     # 8 tiles along D
    NS = S // P     # 16 tiles along S

    with tile.TileContext(nc) as tc:
        with ExitStack() as ctx:
            pool = ctx.enter_context(tc.tile_pool(name="main", bufs=1))
            psum = ctx.enter_context(tc.tile_pool(name="psum", bufs=1, space="PSUM"))

            def ptile(shape, name, tag, dt):
                return pool.tile(list(shape), dt, name=name, tag=tag, bufs=1)

            def qbank(i, name, shape=(P, 1024)):
                # 4 PSUM tags x 2 banks each = all 8 banks
                return psum.tile(list(shape), FP, name=name, tag=f"q{i}", bufs=1)

            # ---- DMAs: phase-1 inputs first (interleaved per dd so the PE
            #      starts after ~0.5 MB), then keys' second half, then the
            #      fp16 output-path operands.
            w2t8 = [ptile([P, 2, D], f"w2t8_{dd}", f"w2t8_{dd}", F8)
                    for dd in range(4)]
            xall8 = [ptile([P, 2, S], f"xall8_{dd}", f"xall8_{dd}", F8)
                     for dd in range(4)]
            for dd in range(4):
                nc.sync.dma_start(w2t8[dd][:], w2t8_d[dd])
                nc.sync.dma_start(xall8[dd][:, :, 0:SQ], xall8_d[dd][:, :, 0:SQ])
            for dd in range(4):
                nc.sync.dma_start(xall8[dd][:, :, SQ:S], xall8_d[dd][:, :, SQ:S])
            ones_sb = ptile([P, 1], "ones", "ones", F32R)
            nc.sync.dma_start(ones_sb[:], ones_d[:, :])
            onesr_sb = ptile([1, P], "onesr", "onesr", F32R)
            nc.sync.dma_start(onesr_sb[:], onesr_d[:, :])
            biasc_sb = ptile([P, 8], "biasc", "biasc", FP)
            nc.sync.dma_start(biasc_sb[:], biasc_d[:, :])
            xrow = [ptile([P, D], f"xrow{sk}", f"xrow{sk}", F16)
                    for sk in range(NS)]
            for sk in range(NS):
                nc.sync.dma_start(xrow[sk][:], xrow_d[sk])
            w3t = [ptile([P, D], f"w3t{d}", f"w3t{d}", F16) for d in range(ND)]
            for d in range(ND):
                nc.sync.dma_start(w3t[d][:], w3t_d[d])

            # ---- phase 1: G[d][128, 1024q], fp8 DoubleRow over e-pairs ----
            g8 = [ptile([P, 2, SQ], f"g8_{dd}", f"g8_{dd}", F8) for dd in range(4)]
            for gp in range(2):
                pgs = [qbank(i, f"pg_{gp}_{i}") for i in range(4)]
                for ee in range(4):
                    for i in range(4):
                        d = gp * 4 + i
                        lt = w2t8[ee][:, :, d * P:(d + 1) * P]
                        for qh in range(2):
                            nc.tensor.matmul(
                                pgs[i][:, qh * 512:(qh + 1) * 512], lt,
                                xall8[ee][:, :, qh * 512:(qh + 1) * 512],
                                start=(ee == 0), stop=(ee == 3), perf_mode=DR)
                # split the PSUM->fp8 casts across DVE and ACT so the
                # inter-phase bubble halves
                for i in range(4):
                    d = gp * 4 + i
                    dst = g8[d // 2][:, d % 2, :]
                    if i % 2 == 0:
                        nc.vector.tensor_copy(dst, pgs[i][:])
                    else:
                        nc.scalar.activation(dst, pgs[i][:], Copy)

            # ---- phase 2: scoresT (fp8 DR) -> exp -> expT fp16; rowsum acc --
            expT = [ptile([P, SQ], f"expT{sk}", f"expT{sk}", F16)
                    for sk in range(NS)]
            acc_sb = ptile([P, SQ], "acc_sb", "acc_sb", F32R)
            for sk in range(NS):
                psc = qbank(sk % 2, f"psc_{sk}")
                for ee in range(4):
                    lt = xall8[ee][:, :, sk * P:(sk + 1) * P]
                    for qh in range(2):
                        nc.tensor.matmul(
                            psc[:, qh * 512:(qh + 1) * 512], lt,
                            g8[ee][:, :, qh * 512:(qh + 1) * 512],
                            start=(ee == 0), stop=(ee == 3), perf_mode=DR)
                for qh in range(2):
                    nc.scalar.activation(expT[sk][:, qh * 512:(qh + 1) * 512],
                                         psc[:, qh * 512:(qh + 1) * 512],
                                         Exp, scale=ESCALE)
                if sk == 0:
                    nc.vector.tensor_copy(acc_sb[:], expT[0][:])
                else:
                    nc.vector.tensor_tensor(acc_sb[:], acc_sb[:], expT[sk][:],
                                            ADD)

            # ---- phase 3: rowsum -> 1/r ----
            pc = qbank(2, "pcs", shape=(1, 1024))
            pc0, pc1 = pc[0:1, 0:512], pc[0:1, 512:1024]
            nc.tensor.matmul(pc0, ones_sb[:], acc_sb[:, 0:512],
                             start=True, stop=True)
            nc.tensor.matmul(pc1, ones_sb[:], acc_sb[:, 512:1024],
                             start=True, stop=True)
            r_row = ptile([1, SQ], "r_row", "r_row", F32R)
            with nc.allow_low_precision(
                    "f32r storage is fp32 bits; only the K=1 broadcast "
                    "matmul reads it at TF32 precision (~5e-4)"):
                nc.vector.reciprocal(r_row[0:1, 0:512], pc0)
                nc.vector.reciprocal(r_row[0:1, 512:1024], pc1)
            rb_sb = ptile([P, SQ], "rb_sb", "rb_sb", FP)

            # ---- phase 4: zT[f][128d, 1024q] = sum_sk xrow[sk][:,f].T @
            #      expT[sk], fp16, 2-tile f-passes with rotating PSUM tags.
            #      The r broadcast matmuls are emitted after the first group
            #      so the in-order PE never waits on the reciprocal. ----
            zt_sb = [ptile([P, SQ], f"zt{f}", f"zt{f}", F16) for f in range(ND)]
            qrot = 0
            for fg in range(4):
                otp = [qbank((qrot + i) % 4, f"ot_{fg}_{i}") for i in range(2)]
                qrot = (qrot + 2) % 4
                for sk in range(NS):
                    for i in range(2):
                        f = fg * 2 + i
                        lt = xrow[sk][:, f * P:(f + 1) * P]
                        for qh in range(2):
                            nc.tensor.matmul(
                                otp[i][:, qh * 512:(qh + 1) * 512], lt,
                                expT[sk][:, qh * 512:(qh + 1) * 512],
                                start=(sk == 0), stop=(sk == NS - 1))
                if fg == 0:
                    # broadcast r_row across partitions via K=1 f32r matmul
                    prb = qbank(3, "prb")
                    nc.tensor.matmul(prb[:, 0:512], onesr_sb[:],
                                     r_row[0:1, 0:512], start=True, stop=True)
                    nc.tensor.matmul(prb[:, 512:1024], onesr_sb[:],
                                     r_row[0:1, 512:1024], start=True, stop=True)
                    nc.vector.tensor_copy(rb_sb[:, 0:512], prb[:, 0:512])
                    nc.vector.tensor_copy(rb_sb[:, 512:1024], prb[:, 512:1024])
                for i in range(2):
                    f = fg * 2 + i
                    nc.vector.tensor_copy(zt_sb[f][:], otp[i][:])

            # ---- phase 5: yT[e][128, 1024q] = sum_d w3t[d][:,e].T @ zt_sb[d];
            #      normalize + bias on DVE; DMA out. ----
            ysb = [ptile([P, SQ], f"ysb_{j}", f"ysb_{j}", FP) for j in range(4)]
            for eg in range(4):
                oyp = [qbank((qrot + i) % 4, f"oy_{eg}_{i}") for i in range(2)]
                qrot = (qrot + 2) % 4
                for d in range(ND):
                    for i in range(2):
                        e = eg * 2 + i
                        lt = w3t[d][:, e * P:(e + 1) * P]
                        for qh in range(2):
                            nc.tensor.matmul(
                                oyp[i][:, qh * 512:(qh + 1) * 512], lt,
                                zt_sb[d][:, qh * 512:(qh + 1) * 512],
                                start=(d == 0), stop=(d == ND - 1))
                for i in range(2):
                    e = eg * 2 + i
                    yt = ysb[(eg * 2 + i) % 4]
                    nc.vector.tensor_tensor(yt[:], oyp[i][:], rb_sb[:], MUL)
                    nc.vector.tensor_scalar_add(yt[:], yt[:],
                                                biasc_sb[:, e:e + 1])
                    nc.sync.dma_start(yt_d[e * P:(e + 1) * P, :], yt[:])

    nc.compile()
    return nc


def _get_nc():
    if "nc" not in _CACHED:
        _CACHED["nc"] = _build_nc()
    return _CACHED["nc"]


def _fp8(a):
    return np.clip(a, -240.0, 240.0).astype(ml_dtypes.float8_e4m3fn)


def make_in_maps(x, w_qkv, w_proj, b_proj):
    wq = w_qkv[0:D]
    wk = w_qkv[D:2 * D]
    wv = w_qkv[2 * D:3 * D]
    w2 = wk.T @ wq                   # scoresT = Xk W2 Xq^T
    w3 = w_proj @ wv                 # y = (E X) W3^T / rowsum
    # W2'^T pair tiles [4, 128, 2, 1024]
    w2tA = np.ascontiguousarray((ALPHA * w2).T)
    w2t8 = _fp8(w2tA.reshape(4, 2, P, D).transpose(0, 2, 1, 3))
    w2t8 = np.ascontiguousarray(w2t8)
    w3t16 = np.ascontiguousarray(w3.T.astype(np.float16).reshape(8, P, D))
    biasc = np.ascontiguousarray(b_proj.reshape(8, P).T)
    ones = np.ones((P, 1), dtype=np.float32)
    onesr = np.ones((1, P), dtype=np.float32)
    in_maps = []
    for c in range(8):
        b, h = c // 2, c % 2
        own = x[b, h * SQ:(h + 1) * SQ]       # [1024, D] our queries
        other = x[b, (1 - h) * SQ:(2 - h) * SQ]
        xp = np.concatenate([own, other], axis=0)       # [2048, D] own-first
        xt = xp.T                                        # [D, 2048]
        xall8 = _fp8(xt.reshape(4, 2, P, S).transpose(0, 2, 1, 3))
        in_maps.append({
            "xall8": np.ascontiguousarray(xall8),
            "w2t8": w2t8,
            "xrow": np.ascontiguousarray(xp.astype(np.float16)
                                         .reshape(NSZ, P, D)),
            "w3t": w3t16,
            "biasc": biasc, "ones": ones, "onesr": onesr,
        })
    return in_maps


NSZ = S // P


def gather_out(results):
    out = np.empty((4, S, D), dtype=np.float32)
    for c in range(8):
        b, h = c // 2, c % 2
        out[b, h * SQ:(h + 1) * SQ] = results[c]["yt"].T
    return out


def kernel(x, w_qkv, w_proj, b_proj):
    from concourse import bass_utils
    nc = _get_nc()
    in_maps = make_in_maps(np.asarray(x, dtype=np.float32),
                           np.asarray(w_qkv, dtype=np.float32),
                           np.asarray(w_proj, dtype=np.float32),
                           np.asarray(b_proj, dtype=np.float32))
    res = bass_utils.run_bass_kernel_spmd(nc, in_maps, list(range(8))).results
    return gather_out(res)


# revision 4
# speedup vs baseline: 1.5863x; 1.0014x over previous
"""Trainium2 Bass kernel for nn_AttentionBlock (B=4, S=2048, D=1024, single head).

Sharding: 8 cores = 4 batches x 2 query-halves; each core owns 1024 queries
of one batch and returns that [1024, 1024] slice of the output (transposed;
the host gather transposes it back).

Algebraic restructure (all folds host-side, x-independent):
    W2 = Wk^T @ Wq   ->  scoresT = Xk W2 Xq^T   (Q, K never materialized)
    y  = softmax(s) @ V @ Wp^T = (E @ X) @ W3^T / rowsum,  W3 = Wp @ Wv
The projection is applied AFTER the attention-weighted sum of X (z = E @ X,
then y = z @ W3^T): z is per-core-unique while VP = X @ W3^T would be
recomputed by both cores of a batch.

Precision: the scores path (G = W2' Xq^T and scoresT = Xk G) runs in
fp8 e4m3 with DoubleRow perf mode (2 K-tiles per matmul); softmax is
insensitive to ~1% score noise. W2 is pre-scaled by ALPHA=64 so its entries
are normal-range in fp8; the exp activation scale absorbs 1/ALPHA. The
output path (zT = X^T E^T, yT = W3 zT) uses fp16 operands with fp32 PSUM.

Phases (PE-major, in-order engines; PSUM = 4 double-bank tags q0..q3):
    1. G[d][128, 1024q] = sum_e W2'^T[e-pairs] @ Xq^T[e-pairs]  (fp8 DR)
       in groups of 4/2/2 PSUM tags so the fp8 casts of one group overlap
       the next group's matmuls (casts split across DVE and ACT)
    2. per key tile sk: scoresT = sum_d Xk^T[d-pairs] @ g8      (fp8 DR)
       -> exp (ACT, scale=SCALE/ALPHA) -> expT fp16; DVE running rowsum
    3. rowsum via ones-column matmul -> reciprocal -> r broadcast via
       K=1 f32r matmul (emitted inside phase 4's first group)
    4. zT[f][128d, 1024q] = sum_sk Xrow[sk,f-cols] @ expT[sk]   (fp16)
       in 2-tile f-passes with rotating PSUM tags; the softmax
       normalization (* rb) is folded into the PSUM->SBUF drain
    5. yT[e][128, 1024q] = sum_d W3^T[d][e-cols] @ zt_sb[d]     (fp16)
       -> single DVE pass (+ bias) -> DMA out transposed
"""

import numpy as np
import ml_dtypes
from contextlib import ExitStack

D = 1024
S = 2048
SQ = 1024  # queries per core
P = 128
ALPHA = 64.0  # host pre-scale on W2 so fp8 e4m3 stays in normal range
SCALE = float(1.0 / np.sqrt(np.float32(D)).astype(np.float32))
ESCALE = SCALE / ALPHA

_CACHED = {}


def _build_nc():
    import concourse.tile as tile
    from concourse import bacc, mybir

    FP = mybir.dt.float32
    F32R = mybir.dt.float32r
    F16 = mybir.dt.float16
    F8 = mybir.dt.float8e4
    Exp = mybir.ActivationFunctionType.Exp
    Copy = mybir.ActivationFunctionType.Copy
    MUL = mybir.AluOpType.mult
    ADD = mybir.AluOpType.add
    DR = mybir.MatmulPerfMode.DoubleRow

    nc = bacc.Bacc("TRN2", target_bir_lowering=False)
    # x feature-major fp8 pair tiles: [p, dd, i, k] = xp[k, 256*dd+128*i+p]
    xall8_d = nc.declare_dram_parameter("xall8", [P, 4, 2, S], F8, isOutput=False)
    # W2'^T pair tiles: [p, ee, i, d] = ALPHA*w2[d, 256*ee+128*i+p]
    w2t8_d = nc.declare_dram_parameter("w2t8", [P, 4, 2, D], F8, isOutput=False)
    # x row-major fp16: [p, sk, d] = xp[128*sk+p, d]
    xrow_d = nc.declare_dram_parameter("xrow", [P, 16, D], F16, isOutput=False)
    # W3^T fp16: [p, dt, e] = w3[e, 128*dt+p]
    w3t_d = nc.declare_dram_parameter("w3t", [P, 8, D], F16, isOutput=False)
    biasc_d = nc.declare_dram_parameter("biasc", [P, 8], FP, isOutput=False)
    ones_d = nc.declare_dram_parameter("ones", [P, 1], F32R, isOutput=False)
    onesr_d = nc.declare_dram_parameter("onesr", [1, P], F32R, isOutput=False)
    yt_d = nc.declare_dram_parameter("yt", [D, SQ], FP, isOutput=True)

    ND = D // P     # 8 tiles along D
    NS = S // P     # 16 tiles along S

    with tile.TileContext(nc) as tc:
        with ExitStack() as ctx:
            pool = ctx.enter_context(tc.tile_pool(name="main", bufs=1))
            psum = ctx.enter_context(tc.tile_pool(name="psum", bufs=1, space="PSUM"))

            def ptile(shape, name, tag, dt):
                return pool.tile(list(shape), dt, name=name, tag=tag, bufs=1)

            def qbank(i, name, shape=(P, 1024)):
                # 4 PSUM tags x 2 banks each = all 8 banks
                return psum.tile(list(shape), FP, name=name, tag=f"q{i}", bufs=1)

            # ---- DMAs. Phase-1 feed order: per e-pair, the gp0 half of
            #      W2'^T then that pair's own-query columns, so the first
            #      matmul starts after ~384 KB. Everything else is coarse
            #      (fewer dma_starts -> shorter end-of-program semaphore
            #      chain).
            w2t8 = ptile([P, 4, 2, D], "w2t8", "w2t8", F8)
            xall8 = ptile([P, 4, 2, S], "xall8", "xall8", F8)
            for ee in range(4):
                nc.sync.dma_start(w2t8[:, ee, :, 0:512],
                                  w2t8_d[:, ee, :, 0:512])
                nc.sync.dma_start(xall8[:, ee, :, 0:SQ],
                                  xall8_d[:, ee, :, 0:SQ])
            nc.sync.dma_start(w2t8[:, :, :, 512:1024],
                              w2t8_d[:, :, :, 512:1024])
            nc.sync.dma_start(xall8[:, :, :, SQ:S], xall8_d[:, :, :, SQ:S])
            ones_sb = ptile([P, 1], "ones", "ones", F32R)
            nc.sync.dma_start(ones_sb[:], ones_d[:, :])
            onesr_sb = ptile([1, P], "onesr", "onesr", F32R)
            nc.sync.dma_start(onesr_sb[:], onesr_d[:, :])
            biasc_sb = ptile([P, 8], "biasc", "biasc", FP)
            nc.sync.dma_start(biasc_sb[:], biasc_d[:, :])
            xrow = ptile([P, 16, D], "xrow", "xrow", F16)
            for c4 in range(4):
                nc.sync.dma_start(xrow[:, 4 * c4:4 * c4 + 4, :],
                                  xrow_d[:, 4 * c4:4 * c4 + 4, :])
            w3t = ptile([P, 8, D], "w3t", "w3t", F16)
            for c4 in range(2):
                nc.sync.dma_start(w3t[:, 4 * c4:4 * c4 + 4, :],
                                  w3t_d[:, 4 * c4:4 * c4 + 4, :])

            # ---- phase 1: G[d][128, 1024q], fp8 DoubleRow over e-pairs.
            #      Groups of 4/2/2 d-tiles; each group's casts overlap the
            #      next group's matmuls. ----
            g8 = [ptile([P, 2, SQ], f"g8_{dd}", f"g8_{dd}", F8) for dd in range(4)]
            for gbase, gcnt in [(0, 4), (4, 2), (6, 2)]:
                pgs = [qbank(i, f"pg_{gbase}_{i}") for i in range(gcnt)]
                for ee in range(4):
                    for i in range(gcnt):
                        d = gbase + i
                        lt = w2t8[:, ee, :, d * P:(d + 1) * P]
                        for qh in range(2):
                            nc.tensor.matmul(
                                pgs[i][:, qh * 512:(qh + 1) * 512], lt,
                                xall8[:, ee, :, qh * 512:(qh + 1) * 512],
                                start=(ee == 0), stop=(ee == 3), perf_mode=DR)
                # split the PSUM->fp8 casts across DVE and ACT
                for i in range(gcnt):
                    d = gbase + i
                    dst = g8[d // 2][:, d % 2, :]
                    if i % 2 == 0:
                        nc.vector.tensor_copy(dst, pgs[i][:])
                    else:
                        nc.scalar.activation(dst, pgs[i][:], Copy)

            # ---- phase 2: scoresT (fp8 DR) -> exp -> expT fp16; rowsum acc --
            expT = [ptile([P, SQ], f"expT{sk}", f"expT{sk}", F16)
                    for sk in range(NS)]
            acc_sb = ptile([P, SQ], "acc_sb", "acc_sb", F32R)
            for sk in range(NS):
                psc = qbank(sk % 2, f"psc_{sk}")
                for ee in range(4):
                    lt = xall8[:, ee, :, sk * P:(sk + 1) * P]
                    for qh in range(2):
                        nc.tensor.matmul(
                            psc[:, qh * 512:(qh + 1) * 512], lt,
                            g8[ee][:, :, qh * 512:(qh + 1) * 512],
                            start=(ee == 0), stop=(ee == 3), perf_mode=DR)
                for qh in range(2):
                    nc.scalar.activation(expT[sk][:, qh * 512:(qh + 1) * 512],
                                         psc[:, qh * 512:(qh + 1) * 512],
                                         Exp, scale=ESCALE)
                if sk == 0:
                    nc.vector.tensor_copy(acc_sb[:], expT[0][:])
                else:
                    nc.vector.tensor_tensor(acc_sb[:], acc_sb[:], expT[sk][:],
                                            ADD)

            # ---- phase 3: rowsum -> 1/r ----
            pc = qbank(2, "pcs", shape=(1, 1024))
            pc0, pc1 = pc[0:1, 0:512], pc[0:1, 512:1024]
            nc.tensor.matmul(pc0, ones_sb[:], acc_sb[:, 0:512],
                             start=True, stop=True)
            nc.tensor.matmul(pc1, ones_sb[:], acc_sb[:, 512:1024],
                             start=True, stop=True)
            r_row = ptile([1, SQ], "r_row", "r_row", F32R)
            with nc.allow_low_precision(
                    "f32r storage is fp32 bits; only the K=1 broadcast "
                    "matmul reads it at TF32 precision (~5e-4)"):
                nc.vector.reciprocal(r_row[0:1, 0:512], pc0)
                nc.vector.reciprocal(r_row[0:1, 512:1024], pc1)
            rb_sb = ptile([P, SQ], "rb_sb", "rb_sb", FP)

            # ---- phase 4: zT[f][128d, 1024q] = sum_sk xrow[sk][:,f].T @
            #      expT[sk], fp16, 2-tile f-passes with rotating PSUM tags.
            #      The drain applies the softmax normalization (* rb). The r
            #      broadcast matmuls are emitted after the first group so the
            #      in-order PE never waits on the reciprocal. ----
            zt_sb = [ptile([P, SQ], f"zt{f}", f"zt{f}", F16) for f in range(ND)]
            qrot = 0
            for fg in range(4):
                otp = [qbank((qrot + i) % 4, f"ot_{fg}_{i}") for i in range(2)]
                qrot = (qrot + 2) % 4
                for sk in range(NS):
                    for i in range(2):
                        f = fg * 2 + i
                        lt = xrow[:, sk, f * P:(f + 1) * P]
                        for qh in range(2):
                            nc.tensor.matmul(
                                otp[i][:, qh * 512:(qh + 1) * 512], lt,
                                expT[sk][:, qh * 512:(qh + 1) * 512],
                                start=(sk == 0), stop=(sk == NS - 1))
                if fg == 0:
                    # broadcast r_row across partitions via K=1 f32r matmul
                    prb = qbank(3, "prb")
                    nc.tensor.matmul(prb[:, 0:512], onesr_sb[:],
                                     r_row[0:1, 0:512], start=True, stop=True)
                    nc.tensor.matmul(prb[:, 512:1024], onesr_sb[:],
                                     r_row[0:1, 512:1024], start=True, stop=True)
                    nc.vector.tensor_copy(rb_sb[:, 0:512], prb[:, 0:512])
                    nc.vector.tensor_copy(rb_sb[:, 512:1024], prb[:, 512:1024])
                for i in range(2):
                    f = fg * 2 + i
                    nc.vector.tensor_tensor(zt_sb[f][:], otp[i][:], rb_sb[:],
                                            MUL)

            # ---- phase 5: yT[e][128, 1024q] = sum_d w3t[d][:,e].T @ zt_sb[d];
            #      single DVE pass adds the bias; DMA out. ----
            ysb = [ptile([P, SQ], f"ysb_{j}", f"ysb_{j}", FP) for j in range(4)]
            for eg in range(4):
                oyp = [qbank((qrot + i) % 4, f"oy_{eg}_{i}") for i in range(2)]
                qrot = (qrot + 2) % 4
                for d in range(ND):
                    for i in range(2):
                        e = eg * 2 + i
                        lt = w3t[:, d, e * P:(e + 1) * P]
                        for qh in range(2):
                            nc.tensor.matmul(
                                oyp[i][:, qh * 512:(qh + 1) * 512], lt,
                                zt_sb[d][:, qh * 512:(qh + 1) * 512],
                                start=(d == 0), stop=(d == ND - 1))
                for i in range(2):
                    e = eg * 2 + i
                    yt = ysb[(eg * 2 + i) % 4]
                    nc.vector.tensor_scalar_add(yt[:], oyp[i][:],
                                                biasc_sb[:, e:e + 1])
                    nc.sync.dma_start(yt_d[e * P:(e + 1) * P, :], yt[:])

    nc.compile()
    return nc


def _get_nc():
    if "nc" not in _CACHED:
        _CACHED["nc"] = _build_nc()
    return _CACHED["nc"]


def _fp8(a):
    return np.clip(a, -240.0, 240.0).astype(ml_dtypes.float8_e4m3fn)


def make_in_maps(x, w_qkv, w_proj, b_proj):
    wq = w_qkv[0:D]
    wk = w_qkv[D:2 * D]
    wv = w_qkv[2 * D:3 * D]
    w2 = wk.T @ wq                   # scoresT = Xk W2 Xq^T
    w3 = w_proj @ wv                 # y = (E X) W3^T / rowsum
    # W2'^T pair tiles [128, 4, 2, 1024]: [p, ee, i, d] = ALPHA*w2[d, .]
    w2tA = np.ascontiguousarray((ALPHA * w2).T)      # [e, d]
    w2t8 = _fp8(w2tA.reshape(4, 2, P, D).transpose(2, 0, 1, 3))
    w2t8 = np.ascontiguousarray(w2t8)
    w3t16 = np.ascontiguousarray(
        w3.T.astype(np.float16).reshape(8, P, D).transpose(1, 0, 2))
    biasc = np.ascontiguousarray(b_proj.reshape(8, P).T)
    ones = np.ones((P, 1), dtype=np.float32)
    onesr = np.ones((1, P), dtype=np.float32)
    in_maps = []
    for c in range(8):
        b, h = c // 2, c % 2
        own = x[b, h * SQ:(h + 1) * SQ]       # [1024, D] our queries
        other = x[b, (1 - h) * SQ:(2 - h) * SQ]
        xp = np.concatenate([own, other], axis=0)       # [2048, D] own-first
        xt = xp.T                                        # [D, 2048]
        xall8 = _fp8(xt.reshape(4, 2, P, S).transpose(2, 0, 1, 3))
        xrow16 = xp.astype(np.float16).reshape(16, P, D).transpose(1, 0, 2)
        in_maps.append({
            "xall8": np.ascontiguousarray(xall8),
            "w2t8": w2t8,
            "xrow": np.ascontiguousarray(xrow16),
            "w3t": w3t16,
            "biasc": biasc, "ones": ones, "onesr": onesr,
        })
    return in_maps


def gather_out(results):
    out = np.empty((4, S, D), dtype=np.float32)
    for c in range(8):
        b, h = c // 2, c % 2
        out[b, h * SQ:(h + 1) * SQ] = results[c]["yt"].T
    return out


def kernel(x, w_qkv, w_proj, b_proj):
    from concourse import bass_utils
    nc = _get_nc()
    in_maps = make_in_maps(np.asarray(x, dtype=np.float32),
                           np.asarray(w_qkv, dtype=np.float32),
                           np.asarray(w_proj, dtype=np.float32),
                           np.asarray(b_proj, dtype=np.float32))
    res = bass_utils.run_bass_kernel_spmd(nc, in_maps, list(range(8))).results
    return gather_out(res)
